# revision 14
# baseline (speedup 1.0000x reference)
"""CondInst fused kernel for 8 Trainium2 NeuronCores (v2).

The reference output depends only on batch element 0 of cnn_feature:
  - params are gathered from ctrl[0] at detection centers
  - feats is a broadcast of mask_feats[0]
so the tower/controller work for batches 1..3 is dead code, and the
controller conv is only needed at the 100 detection positions.

Strategy (embarrassingly parallel, no collectives):
  - Spatially shard batch-0 across the 8 cores: 20 output rows each,
    with a 4-row halo on the input so the 4 chained 3x3 convs need no
    inter-core exchange.
  - BN scale is folded into the conv weights host-side; the BN+relu
    drain is then a single bias+relu op.  Out-of-image halo rows are
    zeroed by a -1e30 bias sentinel (relu(x - 1e30) == 0).
  - The controller conv at the 100 detection points is a tiny matmul on
    host-gathered 3x3 patches (contract dim 1152), computed on-device.
  - conv layer 4 + 1x1 proj + the whole dynamic mask head are pipelined
    per 3-row chunk, so the head overlaps the final conv layer instead
    of running after the full tower.
  - Head layer-2 results for 3 groups land in one psum tile at
    partition offsets {0,32,64}; a single bias-add drain per tile per
    chunk replaces per-group drains.
  - All psum->sbuf drains rotate across the Act/DVE/Pool engines.

Layout trick: the controller weight columns are host-permuted so every
on-device rearrangement of the dynamic params is a single strided DMA:
  cols   0:80   w0 stored c'*8+o, c' ordered (feats 0..8, rel-x, rel-y)
  cols  80:144  w1 stored o*8+o'  (per-instance transposed)
  cols 144:152 w2, 152:160 b0, 160:168 b1, 168 b2 (unchanged)
"""

import numpy as np

B, CIN, H, W = 4, 128, 160, 160
K = 100
CH = 8
OUT = 8
STRIDE = 4
EPS = 1e-5
NCORES = 8

ROWS = H // NCORES          # 20 output rows per core
F = W + 2                   # padded row width 162
HALO = 4
RIN = ROWS + 2 * HALO       # 28 input rows per core
P3 = ROWS * F               # 3240 padded pixels per core
NCHUNK = 486                # head / proj free-dim chunk (3 rows)
CONTRACT = CIN * 9          # 1152
GROUPS = [(g * 16, min(16, K - g * 16)) for g in range((K + 15) // 16)]
# psum-tile assignment for head L2: base partition must be 0/32/64
PSTILES = [[0, 1, 2], [3, 4, 5], [6]]

_CACHE = {}


def _param_perm():
    """new param index -> original param index (169,)"""
    perm = np.zeros(169, np.int64)
    corder = [2, 3, 4, 5, 6, 7, 8, 9, 0, 1]
    for cp, c in enumerate(corder):
        for o in range(8):
            perm[cp * 8 + o] = o * 10 + c         # w0
    for o in range(8):
        for o2 in range(8):
            perm[80 + o * 8 + o2] = 80 + o2 * 8 + o   # w1 transposed per-inst
    perm[144:169] = np.arange(144, 169)
    return perm


def _host_prep(inputs):
    """Build the 8 per-core input maps (pure numpy indexing + packing)."""
    import ml_dtypes
    cdt_np = ml_dtypes.bfloat16

    cnn_feature = np.asarray(inputs["cnn_feature"], np.float32)
    tower_w = np.asarray(inputs["tower_w"], np.float32)
    bn_gamma = np.asarray(inputs["bn_gamma"], np.float32)
    bn_beta = np.asarray(inputs["bn_beta"], np.float32)
    bn_mean = np.asarray(inputs["bn_mean"], np.float32)
    bn_var = np.asarray(inputs["bn_var"], np.float32)
    proj_w = np.asarray(inputs["proj_w"], np.float32)
    proj_b = np.asarray(inputs["proj_b"], np.float32)
    ctrl_w = np.asarray(inputs["ctrl_w"], np.float32)
    ctrl_b = np.asarray(inputs["ctrl_b"], np.float32)
    detection = np.asarray(inputs["detection"])

    x0 = cnn_feature[0]                                   # [128, 160, 160]

    # BN scale folded into the conv weights; shift stays as bias
    inv = bn_gamma / np.sqrt(bn_var + EPS)                # [4, 128]
    shift = bn_beta - bn_mean * inv                       # [4, 128]

    # tower weights as lhsT per tap: twT[i*9+ky*3+kx] = (W[i]*inv[i]).T
    twT = (tower_w.transpose(0, 3, 4, 2, 1)               # [i, ky, kx, c, o]
           * inv[:, None, None, None, :]).reshape(36, 128, 128)
    twT = np.ascontiguousarray(twT).astype(cdt_np)

    # controller weights, column-permuted, +bias row, padded to 1280 contract
    perm = _param_perm()
    cw_flat = ctrl_w.reshape(169, CONTRACT)
    cwT = np.zeros((1280, 169), np.float32)
    cwT[:CONTRACT, :] = cw_flat[perm].T
    cwT[CONTRACT, :] = ctrl_b[perm]

    # patches at detection centers, transposed, +ones row; fused with cwT
    xs = detection[:, 0].astype(np.int64)
    ys = detection[:, 1].astype(np.int64)
    xpad2 = np.pad(x0, ((0, 0), (1, 1), (1, 1)))
    pcw = np.zeros((1280, K + 169), np.float32)
    for k in range(K):
        pcw[:CONTRACT, k] = xpad2[:, ys[k]:ys[k] + 3, xs[k]:xs[k] + 3].ravel()
    pcw[CONTRACT, :K] = 1.0
    pcw[:, K:] = cwT

    # detection centers replicated 8x along partitions, per 16-instance group:
    # detfan[kl*8+o, g] = 4*x_{16g+kl} (cols 0..6), 4*y (cols 7..13)
    det4 = detection.astype(np.float32) * STRIDE
    detfan = np.zeros((128, 14), np.float32)
    for g, (k0, gsz) in enumerate(GROUPS):
        for kl in range(gsz):
            detfan[kl * 8:kl * 8 + 8, g] = det4[k0 + kl, 0]
            detfan[kl * 8:kl * 8 + 8, 7 + g] = det4[k0 + kl, 1]

    onesbd = np.zeros((128, 16), np.float32)
    for kl in range(16):
        onesbd[kl * 8:kl * 8 + 8, kl] = 1.0

    projT = np.ascontiguousarray(proj_w.T).astype(cdt_np)  # [128, 8]
    projb = proj_b.reshape(8, 1).astype(np.float32)

    # per-core padded input slices
    xpad_rows = np.zeros((128, H + 2 * HALO, F), np.float32)
    xpad_rows[:, HALO:HALO + H, 1:161] = x0
    xpad_rows = xpad_rows.astype(cdt_np)

    shared = dict(twT=twT, pcw=pcw.astype(cdt_np), detfan=detfan,
                  onesbd=onesbd.astype(cdt_np), projT=projT, projb=projb)

    in_maps = []
    for c in range(NCORES):
        xin = np.ascontiguousarray(xpad_rows[:, ROWS * c:ROWS * c + RIN, :])

        # bnv[ch, i*3+region] = shift; -1e30 for out-of-image regions
        bnv = np.zeros((128, 12), np.float32)
        for i in range(4):
            bnv[:, i * 3 + 1] = shift[i]
            bnv[:, i * 3 + 0] = shift[i] if c != 0 else -1e30
            bnv[:, i * 3 + 2] = shift[i] if c != NCORES - 1 else -1e30

        grid = np.zeros((2, ROWS, F), np.float32)
        gxrow = -(np.arange(W, dtype=np.float32) * STRIDE + STRIDE // 2)
        gyv = -(np.arange(ROWS * c, ROWS * c + ROWS, dtype=np.float32) * STRIDE
                + STRIDE // 2)
        grid[0, :, 1:161] = gxrow[None, :]
        grid[1, :, 1:161] = gyv[:, None]

        in_maps.append(dict(shared, xin=xin, bnv=bnv,
                            grid=grid.reshape(2, ROWS * F).astype(cdt_np)))
    return in_maps


def _build_program():
    from contextlib import ExitStack
    import concourse.bass as bass
    import concourse.tile as tile
    from concourse import bacc, mybir

    f32 = mybir.dt.float32
    cdt = mybir.dt.bfloat16
    Relu = mybir.ActivationFunctionType.Relu
    Ident = mybir.ActivationFunctionType.Identity
    Alu = mybir.AluOpType

    def man_ap(base, rel_off, dims):
        """manual flat-element AP: dims = [[stride, count], ...]"""
        return bass.AP(tensor=base.tensor, offset=base.offset + rel_off,
                       ap=[list(d) for d in dims])

    nc = bacc.Bacc("TRN2", target_bir_lowering=False, debug=False,
                   enable_asserts=False, detect_race_conditions=False)

    xin_d = nc.dram_tensor("xin", [128, RIN, F], cdt, kind="ExternalInput")
    twT_d = nc.dram_tensor("twT", [36, 128, 128], cdt, kind="ExternalInput")
    bnv_d = nc.dram_tensor("bnv", [128, 12], f32, kind="ExternalInput")
    grid_d = nc.dram_tensor("grid", [2, P3], cdt, kind="ExternalInput")
    pcw_d = nc.dram_tensor("pcw", [1280, K + 169], cdt, kind="ExternalInput")
    detfan_d = nc.dram_tensor("detfan", [128, 14], f32, kind="ExternalInput")
    onesbd_d = nc.dram_tensor("onesbd", [128, 16], cdt, kind="ExternalInput")
    projT_d = nc.dram_tensor("projT", [128, 8], cdt, kind="ExternalInput")
    projb_d = nc.dram_tensor("projb", [8, 1], f32, kind="ExternalInput")
    out_d = nc.dram_tensor("out", [K, ROWS, W], f32, kind="ExternalOutput")

    NC2 = K + 169   # pcw row width (269)

    # engine handles for drain rotation
    def drain_relu(which, out, in_, bias):
        if which % 2 == 0:
            nc.scalar.activation(out=out, in_=in_, func=Relu, bias=bias)
        else:
            nc.vector.tensor_scalar(out=out, in0=in_, scalar1=bias,
                                    scalar2=0.0, op0=Alu.add, op1=Alu.max)

    def drain_add(which, out, in_, bias):
        if which % 2 == 0:
            nc.scalar.activation(out=out, in_=in_, func=Ident, bias=bias)
        else:
            nc.vector.tensor_scalar(out=out, in0=in_, scalar1=bias,
                                    scalar2=None, op0=Alu.add)

    with tile.TileContext(nc) as tc, ExitStack() as ctx:
        const = ctx.enter_context(tc.tile_pool(name="const", bufs=1))
        prep = ctx.enter_context(tc.tile_pool(name="prep", bufs=1))
        convp = ctx.enter_context(tc.tile_pool(name="conv", bufs=1))

        # ---------------- tiles ----------------
        xbuf = convp.tile([128, RIN * F + 2], cdt, tag="xbuf")
        tw_all = const.tile([128, 36 * 128], cdt)
        tw_sb = [tw_all[:, t * 128:(t + 1) * 128] for t in range(36)]
        bnv_sb = const.tile([128, 12], f32)
        hbase = const.tile([10, P3], cdt)
        detfan_sb = const.tile([128, 14], f32)
        onesbd_sb = const.tile([128, 16], cdt)
        projT_sb = const.tile([128, 8], cdt)
        projb_sb = const.tile([8, 1], f32)
        pc_all = const.tile([128, 10 * NC2], cdt)

        p2 = prep.tile([K, 169], cdt)
        p2d = prep.tile([K, 169], cdt, space="DRAM", name="p2d")
        l0 = prep.tile([10, 8 * K], cdt)
        bd_all = prep.tile([128, len(GROUPS) * 128], cdt)
        bd1 = [bd_all[:, g * 128:g * 128 + gsz * 8]
               for g, (k0, gsz) in enumerate(GROUPS)]
        fanstage = prep.tile([128, 35], cdt)
        fans = prep.tile([128, 35], f32)
        beta0fan = prep.tile([128, 7], f32)
        tmpf = prep.tile([128, 7], f32)
        bd2 = [prep.tile([gsz * 8, 16], cdt, tag=f"bdw2_{g}", name=f"bdw2_{g}")
               for g, (k0, gsz) in enumerate(GROUPS)]
        b2col = [prep.tile([80, 1], f32, tag=f"b2c{t}", name=f"b2c{t}")
                 for t in range(3)]
        outg = [prep.tile([80, P3], f32, tag="outgA", name="outgA"),
                prep.tile([80, P3], f32, tag="outgB", name="outgB"),
                prep.tile([32, P3], f32, tag="outgC", name="outgC")]

        # ---------------- memsets (DVE, before dependent work) -----------
        nc.vector.memset(xbuf[:, 0:1], 0.0)
        nc.vector.memset(xbuf[:, 1 + RIN * F:], 0.0)
        nc.vector.memset(bd_all[:], 0.0)
        nc.vector.memset(fanstage[:], 0.0)

        # ---------------- input DMAs, spread across HWDGE queues ---------
        # SP: xin (first 8 rows, then the rest) -> later phase-B + outputs
        nc.sync.dma_start(out=xbuf[:, 1:1 + 8 * F], in_=xin_d[:, 0:8, :])
        nc.sync.dma_start(out=xbuf[:, 1 + 8 * F:1 + RIN * F],
                          in_=xin_d[:, 8:RIN, :])

        # Act: tower weights, layer 1 first
        def _tw_dma(eng, h0, nh):
            eng.dma_start(
                out=man_ap(tw_all[:], 9 * h0 * 128,
                           [[36 * 128, 128], [128, 9 * nh], [1, 128]]),
                in_=man_ap(twT_d[:], 9 * h0 * 128 * 128,
                           [[128, 128], [128 * 128, 9 * nh], [1, 128]]))
        _tw_dma(nc.scalar, 0, 1)

        # Act: bnv (needed by first BN drain), pcw (phase A), then the rest
        nc.scalar.dma_start(out=bnv_sb[:], in_=bnv_d[:])
        nc.scalar.dma_start(
            out=man_ap(pc_all[:], 0,
                       [[10 * NC2, 128], [NC2, 10], [1, NC2]]),
            in_=man_ap(pcw_d[:], 0,
                       [[NC2, 128], [128 * NC2, 10], [1, NC2]]))
        _tw_dma(nc.scalar, 1, 3)
        nc.scalar.dma_start(out=hbase[8:10, :], in_=grid_d[:])
        nc.scalar.dma_start(out=detfan_sb[:], in_=detfan_d[:])
        nc.scalar.dma_start(out=onesbd_sb[:], in_=onesbd_d[:])
        nc.scalar.dma_start(out=projT_sb[:], in_=projT_d[:])
        nc.scalar.dma_start(out=projb_sb[:], in_=projb_d[:])

        # ---------------- conv tower layers 1..3 --------------------------
        # chunk lists: (r0, nr); layer 1 leads with 1-row chunks so the
        # instructions latched at cold PE p-state are small
        def chunk_list(rout, lead1):
            ch = [(r, 1) for r in range(lead1)]
            r = lead1
            while r < rout:
                nr = min(3, rout - r)
                ch.append((r, nr))
                r += nr
            return ch

        layer_chunks = [chunk_list(26, 8), chunk_list(24, 0),
                        chunk_list(22, 0)]

        conv_ctx = ExitStack()
        conv_ps = conv_ctx.enter_context(
            tc.tile_pool(name="conv_ps", bufs=6, space="PSUM"))
        pa_ps = conv_ctx.enter_context(
            tc.tile_pool(name="pa_ps", bufs=1, space="PSUM"))

        rot = 0          # drain-engine rotation counter
        phase_a_done = False
        cur = xbuf
        rcur = RIN
        for i, chunks in enumerate(layer_chunks):
            rout = rcur - 2
            obuf = convp.tile([128, rout * F + 2], cdt, tag=f"c{i}",
                              name=f"c{i}")
            obuf3 = obuf[:, 1:1 + rout * F].rearrange("p (r c) -> p r c", c=F)
            nc.vector.memset(obuf[:, 0:1], 0.0)
            nc.vector.memset(obuf[:, 1 + rout * F:], 0.0)
            nc.vector.memset(obuf3[:, :, 0:1], 0.0)
            nc.vector.memset(obuf3[:, :, 161:162], 0.0)

            T = 3 - i
            bounds = sorted({0, T, rout - T, rout})
            for ci, (r0, nr) in enumerate(chunks):
                ps = conv_ps.tile([128, nr * F], f32, tag="cps", name="cps")
                for t, (ky, kx) in enumerate(
                        (ky, kx) for ky in range(3) for kx in range(3)):
                    off = 1 + (r0 + ky) * F + kx - 1
                    nc.tensor.matmul(
                        ps[:], lhsT=tw_sb[i * 9 + t],
                        rhs=cur[:, off:off + nr * F],
                        start=(t == 0), stop=(t == 8))
                ps3 = ps[:].rearrange("p (r c) -> p r c", c=F)
                # split chunk rows by (top|mid|bot) BN regions
                for rs, re in zip(bounds[:-1], bounds[1:]):
                    a, b = max(rs, r0), min(re, r0 + nr)
                    if a >= b:
                        continue
                    reg = 0 if b <= T else (2 if a >= rout - T else 1)
                    drain_relu(rot,
                               obuf3[:, a:b, 1:161],
                               ps3[:, a - r0:b - r0, 1:161],
                               bnv_sb[:, i * 3 + reg:i * 3 + reg + 1])
                    rot += 1

                # insert phase A early (after a few L1 chunks): the params
                # matmuls + all phase-B builds run under the conv tower
                if i == 0 and ci == 2 and not phase_a_done:
                    phase_a_done = True
                    p2p = pa_ps.tile([K, 169], f32)
                    for h in range(10):
                        nc.tensor.matmul(
                            p2p[:],
                            lhsT=pc_all[:, h * NC2:h * NC2 + K],
                            rhs=pc_all[:, h * NC2 + K:(h + 1) * NC2],
                            start=(h == 0), stop=(h == 9))
                    nc.vector.tensor_copy(p2[:], p2p[:])
                    # round-trip p2 through DRAM: the scatter DMAs below need
                    # partition steps on inner dims, legal only on DRAM APs
                    nc.sync.dma_start(out=p2d[:], in_=p2[:])

                    # ---- phase B: head weight assembly ----
                    # l0[c, k*8+o] = p2[k, c*8+o]   (single strided DMA)
                    nc.sync.dma_start(
                        out=man_ap(l0[:], 0, [[8 * K, 10], [8, K], [1, 8]]),
                        in_=man_ap(p2d[:], 0, [[8, 10], [169, K], [1, 8]]))
                    # bd1: block-diagonal w1 blocks; one DMA per within-group
                    # index kl spanning all groups (dest partition dim 0)
                    BDP = len(GROUPS) * 128    # bd_all row pitch
                    for kl in range(16):
                        ng = sum(1 for k0, gsz in GROUPS if kl < gsz)
                        nc.sync.dma_start(
                            out=man_ap(bd_all[:], (kl * 8) * BDP + kl * 8,
                                       [[BDP, 8], [128, ng], [1, 8]]),
                            in_=man_ap(p2d[:], kl * 169 + 80,
                                       [[8, 8], [16 * 169, ng], [1, 8]]))
                    # fan-out columns (b0, w0x, w0y, b1, w2): Act HWDGE +
                    # a share on the gpsimd SWDGE queue
                    for fi, c0 in enumerate((152, 64, 72, 160, 144)):
                        for g, (k0, gsz) in enumerate(GROUPS):
                            eng = nc.scalar if (fi * 7 + g) % 3 != 2 \
                                else nc.gpsimd
                            eng.dma_start(
                                out=fanstage[0:gsz * 8, fi * 7 + g:
                                             fi * 7 + g + 1],
                                in_=p2[k0:k0 + gsz, c0:c0 + 8])
                    # b2 per psum-tile bias columns (gpsimd: casts bf16->f32)
                    for ti, glist in enumerate(PSTILES):
                        for gi, g in enumerate(glist):
                            k0, gsz = GROUPS[g]
                            nc.gpsimd.dma_start(
                                out=b2col[ti][32 * gi:32 * gi + gsz, 0:1],
                                in_=p2[k0:k0 + gsz, 168:169])
                    # fans -> f32, beta0 = b0 + w0x*4cx + w0y*4cy
                    nc.vector.tensor_copy(fans[:], fanstage[:])
                    nc.vector.tensor_mul(beta0fan[:], fans[:, 7:14],
                                         detfan_sb[:, 0:7])
                    nc.vector.tensor_mul(tmpf[:], fans[:, 14:21],
                                         detfan_sb[:, 7:14])
                    nc.vector.tensor_add(beta0fan[:], beta0fan[:], tmpf[:])
                    nc.vector.tensor_add(beta0fan[:], beta0fan[:],
                                         fans[:, 0:7])
                    # layer-2 weights as block-diag [gp, 16]
                    for g, (k0, gsz) in enumerate(GROUPS):
                        gp = gsz * 8
                        nc.vector.tensor_scalar_mul(
                            bd2[g][:], onesbd_sb[0:gp, :],
                            fans[0:gp, 28 + g:29 + g])
            cur = obuf
            rcur = rout

        conv_ctx.close()

        # ---------------- pipelined conv4 + proj + head -------------------
        c3 = cur                 # [128, 22 rows]
        chunks4 = [(ci, ci * 3, min(3, ROWS - ci * 3))
                   for ci in range((ROWS + 2) // 3)]

        with tc.tile_pool(name="mm0_ps", bufs=2, space="PSUM") as mm0_ps, \
             tc.tile_pool(name="ps1_ps", bufs=2, space="PSUM") as ps1_ps, \
             tc.tile_pool(name="proj_ps", bufs=1, space="PSUM") as proj_ps, \
             tc.tile_pool(name="pso_ps", bufs=1, space="PSUM") as pso_ps, \
             tc.tile_pool(name="c4p", bufs=2) as c4p, \
             tc.tile_pool(name="headp", bufs=2) as headp:
            for ci, r0, nr in chunks4:
                nn = nr * F
                n0 = r0 * F
                # conv4 on this chunk (input c3 rows r0..r0+nr+2)
                ps4 = mm0_ps.tile([128, nn], f32, tag="mm0", name="ps4")
                for t, (ky, kx) in enumerate(
                        (ky, kx) for ky in range(3) for kx in range(3)):
                    off = 1 + (r0 + ky) * F + kx - 1
                    nc.tensor.matmul(
                        ps4[:], lhsT=tw_sb[27 + t],
                        rhs=c3[:, off:off + nn],
                        start=(t == 0), stop=(t == 8))
                c4c = c4p.tile([128, NCHUNK], cdt, tag="c4c", name="c4c")
                drain_relu(rot, c4c[:, 0:nn], ps4[:],
                           bnv_sb[:, 10:11])
                rot += 1
                # proj -> hbase rows 0..8
                pp = proj_ps.tile([8, nn], f32, tag="pps", name="pps")
                nc.tensor.matmul(pp[:], lhsT=projT_sb[:], rhs=c4c[:, 0:nn],
                                 start=True, stop=True)
                drain_add(rot, hbase[0:8, n0:n0 + nn], pp[:],
                          projb_sb[:, 0:1])
                rot += 1

                # head: all 7 groups on this chunk
                pso = [pso_ps.tile([80, nn], f32, tag="psoA", name="psoA"),
                       pso_ps.tile([80, nn], f32, tag="psoB", name="psoB"),
                       pso_ps.tile([32, nn], f32, tag="psoC", name="psoC")]
                for ti, glist in enumerate(PSTILES):
                    for gi, g in enumerate(glist):
                        k0, gsz = GROUPS[g]
                        gp = gsz * 8
                        ps0 = mm0_ps.tile([128, nn], f32, tag="mm0",
                                          name="ps0")
                        nc.tensor.matmul(ps0[0:gp, :],
                                         lhsT=l0[:, 8 * k0:8 * k0 + gp],
                                         rhs=hbase[:, n0:n0 + nn],
                                         start=True, stop=True)
                        h1c = headp.tile([128, NCHUNK], cdt, tag="h1c",
                                         name="h1c")
                        drain_relu(rot, h1c[0:gp, 0:nn], ps0[0:gp, :],
                                   beta0fan[0:gp, g:g + 1])
                        rot += 1
                        ps1 = ps1_ps.tile([128, nn], f32, tag="ps1",
                                          name="ps1")
                        nc.tensor.matmul(ps1[0:gp, :], lhsT=bd1[g][0:gp, :],
                                         rhs=h1c[0:gp, 0:nn],
                                         start=True, stop=True)
                        h2c = headp.tile([128, NCHUNK], cdt, tag="h2c",
                                         name="h2c")
                        drain_relu(rot, h2c[0:gp, 0:nn], ps1[0:gp, :],
                                   fans[0:gp, 21 + g:22 + g])
                        rot += 1
                        nc.tensor.matmul(
                            pso[ti][32 * gi:32 * gi + gsz, :],
                            lhsT=bd2[g][:, 0:gsz], rhs=h2c[0:gp, 0:nn],
                            start=True, stop=True)
                # one bias-add drain per psum tile per chunk
                for ti in range(3):
                    npart = 32 * (len(PSTILES[ti]) - 1) + \
                        GROUPS[PSTILES[ti][-1]][1]
                    drain_add(rot, outg[ti][0:npart, n0:n0 + nn],
                              pso[ti][0:npart, :], b2col[ti][0:npart, 0:1])
                    rot += 1

        # ---------------- output DMAs (spread across queues) --------------
        out_engs = [nc.sync, nc.scalar, nc.sync]
        for g, (k0, gsz) in enumerate(GROUPS):
            ti, gi = divmod(g, 3)
            og3 = outg[ti][:].rearrange("p (r c) -> p r c", c=F)
            out_engs[g % 3].dma_start(
                out=out_d[k0:k0 + gsz, :, :],
                in_=og3[32 * gi:32 * gi + gsz, :, 1:161])

    nc.compile()
    return nc


def _get_program(reps=1):
    if "nc" not in _CACHE:
        _CACHE["nc"] = _build_program()
    return _CACHE["nc"]


def _run(in_maps, trace=False, **kwargs):
    from concourse.bass_utils import run_bass_kernel_spmd
    nc = _get_program()
    return run_bass_kernel_spmd(nc, in_maps, core_ids=list(range(NCORES)),
                                trace=trace, **kwargs)


def kernel(**inputs) -> np.ndarray:
    in_maps = _host_prep(inputs)
    res = _run(in_maps)
    out = np.concatenate([res.results[c]["out"] for c in range(NCORES)], axis=1)
    return out.astype(np.float32)
